# revision 31
# baseline (speedup 1.0000x reference)
"""Trainium2 Bass kernel for nn_Ballistics: per-batch attack/release one-pole
envelope follower y[t] = (1-c)*y[t-1] + c*x[t], c = at if x[t] > y[t-1] else rt.

Algorithm (per core, 8 batch rows):
  Work in the margin variable u[j] = y[j-1] - x[j]:
      u[j+1] = a_j * u[j] - dx[j+1],   dx[j] = x[j] - x[j-1],
      a_j = a_rt if u[j] >= 0 else a_at        (a_* = 1 - coeff)
      y[j] = x[j+1] + u[j+1]
  The branch is sign(u): predicate iteration — freeze predicates from the
  previous iterate, solve the now-linear recurrence exactly with the native
  tensor_tensor_scan (fp32 carry), repeat K=5 times.  The map contracts
  (|a|<1), so the time axis is chunked into 64 chunks x 4096 steps with a
  128-step warm-up whose output is discarded; the host pre-pads x with 1.0
  so y[-1]=1 exactly.  128 partitions = 16 chunks x 8 rows (chunk-major);
  4 groups of 16 chunks, software-pipelined 4-in-flight.

Perf notes (vs the K=6 all-fp32 all-VectorE version, 381803 ns):
  - K=5 suffices for the 2e-2 gate (max/max rel err 8.5e-3 verified by
    numpy emulation of this exact arithmetic, fp16 operands included).
  - All operands fp16: the scan keeps an fp32 carry regardless of operand
    dtype, so only operand/output quantization matters (~1e-3).
  - The host supplies x (fp16) and its first difference dx (fp16) with the
    warmup padding baked in, so each group is exactly 2 DMAs and the
    device never computes dx.
  - The backend rejects tensor_tensor_scan on Pool, so all 5 scans ride
    DVE (its per-group bottleneck); Sign predicates ride Act; affines and
    the final y=x+u add are spread over Act/Pool/DVE (tables below, tuned
    on TimelineSim steady-state reps-delta).
"""
import sys
for p in ("/opt/trn_rl_repo", "/root/.axon_site/_ro/trn_rl_repo"):
    if p not in sys.path:
        sys.path.insert(0, p)

import numpy as np

B, T = 64, 262144
NCORES = 8
RPC = B // NCORES          # rows per core
L = 4096                   # chunk length (output steps per chunk)
W = 128                    # warm-up steps (discarded)
K = 5                      # predicate iterations (number of scans)
C = T // L                 # chunks per row (64)
QP = 128 // RPC            # chunks in flight per row (16)
G = C // QP                # sequential groups (4)
N = L + W + 1              # scan steps per window
NW = N + 1                 # x-window columns per partition
TP = T + W + 2             # padded x row length (W+1 ones + x + one zero)

# Per-iteration engine assignment (len K).  pred engines: "dve" (is_ge ->
# {0,1}, affine uses dlt/aat), "act" (Sign -> {-1,0,1}, affine uses hdl/mid).
# k=0 seed pred comes from dx (is_le 0 on dve / Sign(-dx) on act).
PRED_ENG = ["dve", "act", "act", "act", "act"]
AFF_ENG = ["dve", "act", "act", "pool", "dve"]
SCAN_ENG = ["dve", "dve", "dve", "dve", "dve"]
YADD_ENG = "pool"          # y = xh + u (f16+f16 -> f32)
STARTS = [0, 1, 2, 4]      # wave (within rep period) of each group's k=0
PERIOD = 5                 # rep-to-rep wave stride (4 jobs/wave steady)
WBUFS = 4                  # work-pool buffers (4 groups in flight)
USE_BARRIER = True         # no_sync_barrier between waves

_cache = {}


def _build(reps=1):
    import concourse.bacc as bacc
    import concourse.mybir as mybir
    import concourse.tile as tile
    import concourse.bass as bass

    f32 = mybir.dt.float32
    f16 = mybir.dt.float16
    Alu = mybir.AluOpType
    Act = mybir.ActivationFunctionType

    nc = bacc.Bacc("TRN2", target_bir_lowering=False, debug=False,
                   num_devices=NCORES)
    xh_d = nc.dram_tensor("xh", [RPC, TP], f16, kind="ExternalInput")
    dx_d = nc.dram_tensor("dx", [RPC, TP - 1], f16, kind="ExternalInput")
    mid_d = nc.dram_tensor("mid", [128, 1], f32, kind="ExternalInput")
    hdl_d = nc.dram_tensor("hdl", [128, 1], f32, kind="ExternalInput")
    aat_d = nc.dram_tensor("aat", [128, 1], f32, kind="ExternalInput")
    dlt_d = nc.dram_tensor("dlt", [128, 1], f32, kind="ExternalInput")
    y_d = nc.dram_tensor("y", [RPC, T], f32, kind="ExternalOutput")

    def eng(nc, name):
        return {"dve": nc.vector, "pool": nc.gpsimd}[name]

    with tile.TileContext(nc) as tc:
        with tc.tile_pool(name="cpool", bufs=1) as cpool, \
             tc.tile_pool(name="ypool", bufs=1) as ypool, \
             tc.tile_pool(name="wpool", bufs=WBUFS) as wpool:
            mid_s = cpool.tile([128, 1], f32, tag="mid")
            hdl_s = cpool.tile([128, 1], f32, tag="hdl")
            aat_s = cpool.tile([128, 1], f32, tag="aat")
            dlt_s = cpool.tile([128, 1], f32, tag="dlt")
            nc.sync.dma_start(mid_s[:, :], mid_d.ap()[:, :])
            nc.sync.dma_start(hdl_s[:, :], hdl_d.ap()[:, :])
            nc.sync.dma_start(aat_s[:, :], aat_d.ap()[:, :])
            nc.sync.dma_start(dlt_s[:, :], dlt_d.ap()[:, :])

            def start_group(gr):
                """Two DMAs (xh window, dx window) + ut init.

                Partition layout is chunk-major: partition p = q*RPC + r.
                Host padding makes every group's window APs uniform:
                chunk c reads xh[, c*L : c*L+NW] and dx[, c*L : c*L+N].
                """
                base = gr * QP * L
                xht = wpool.tile([128, NW], f16, tag="xh")
                nc.sync.dma_start(
                    xht[:, :], bass.AP(xh_d, base, [[L, QP], [TP, RPC], [1, NW]]))
                dxt = wpool.tile([128, N], f16, tag="dx")
                nc.sync.dma_start(
                    dxt[:, :],
                    bass.AP(dx_d, base, [[L, QP], [TP - 1, RPC], [1, N]]))
                ut = wpool.tile([128, NW], f16, tag="ut")
                nc.vector.memset(ut[:, 0:1], 0.0)
                return dict(gr=gr, xht=xht, dxt=dxt, ut=ut)

            def emit_job(st, k):
                """pred + affine + scan for iteration k.  bt/att are
                job-scoped scratch (3 rotating slots across all groups)."""
                dxt, ut = st["dxt"], st["ut"]
                bt = wpool.tile([128, NW], f16, tag="bt", bufs=3)
                att = wpool.tile([128, NW], f16, tag="att", bufs=3)
                pe, ae = PRED_ENG[k], AFF_ENG[k]
                if pe == "act":
                    # Sign -> {-1,0,1}; affine scalars (hdl, mid)
                    if k == 0:
                        nc.vector.memset(bt[:, 0:1], 0.0)
                        nc.scalar.activation(bt[:, 1:NW], dxt[:, 0:N],
                                             Act.Sign, scale=-1.0)
                    else:
                        nc.scalar.activation(bt[:, :], ut[:, :], Act.Sign)
                    s1, s2 = hdl_s, mid_s
                else:
                    # is_* -> {0,1}; affine scalars (dlt, aat)
                    if k == 0:
                        nc.vector.memset(bt[:, 0:1], 0.0)
                        eng(nc, pe).tensor_single_scalar(
                            bt[:, 1:NW], dxt[:, 0:N], 0.0, Alu.is_le)
                    else:
                        eng(nc, pe).tensor_single_scalar(
                            bt[:, :], ut[:, :], 0.0, Alu.is_ge)
                    s1, s2 = dlt_s, aat_s
                if ae == "act":
                    nc.scalar.activation(att[:, :], bt[:, :], Act.Identity,
                                         bias=s2[:, 0:1], scale=s1[:, 0:1])
                else:
                    eng(nc, ae).tensor_scalar(
                        att[:, :], bt[:, :], s1[:, 0:1], s2[:, 0:1],
                        Alu.mult, Alu.add)
                eng(nc, SCAN_ENG[k]).tensor_tensor_scan(
                    ut[:, 1:NW], att[:, 0:N], dxt[:, :], 0.0,
                    Alu.mult, Alu.subtract)

            def finish_group(st):
                gr, xht, ut = st["gr"], st["xht"], st["ut"]
                yt = ypool.tile([128, L], f32, tag="yt")
                o = W + 2
                eng(nc, YADD_ENG).tensor_tensor(
                    yt[:, :], xht[:, o:o + L], ut[:, o:o + L], Alu.add)
                nc.sync.dma_start(
                    bass.AP(y_d, gr * QP * L, [[L, QP], [T, RPC], [1, L]]),
                    yt[:, :])

            # Cyclic global-wave emission: rep r's group g starts at wave
            # r*PERIOD + STARTS[g]; consecutive reps' heads/tails overlap
            # (4 groups in flight at any wave -> WBUFS=4).
            start_wave = {}
            for r in range(reps):
                for g in range(G):
                    start_wave[(r, g)] = r * PERIOD + STARTS[g]
            n_waves = (reps - 1) * PERIOD + STARTS[-1] + K + 1
            sts = {}
            for w in range(n_waves):
                # scheduler-only fence: keeps each wave's ops grouped per
                # engine (stops depth-first reordering); no runtime sems.
                if USE_BARRIER:
                    tc.no_sync_barrier()
                # iteration jobs first (latency-critical), oldest group first
                for key in sorted(sts, key=lambda q: -(w - start_wave[q])):
                    k = w - start_wave[key]
                    if 1 <= k < K:
                        emit_job(sts[key], k)
                # then housekeeping: finishes (yadd + DMA out), new starts
                # (each start's k=0 seed job is emitted with it)
                for key, sw in list(start_wave.items()):
                    if w == sw + K and key in sts:
                        finish_group(sts.pop(key))
                for key, sw in start_wave.items():
                    if w == sw:
                        sts[key] = start_group(key[1])
                        emit_job(sts[key], 0)

    nc.compile()
    return nc


def _get_nc(reps=1):
    if reps not in _cache:
        _cache[reps] = _build(reps)
    return _cache[reps]


def _coeffs(z_alpha):
    z = np.asarray(z_alpha, dtype=np.float32)
    ts = (np.float32(1.0) / (np.float32(1.0) + np.exp(-z, dtype=np.float32)))
    at = ts[:, 0].astype(np.float32)
    rt = ts[:, 1].astype(np.float32)
    a_at = (np.float32(1.0) - at).astype(np.float32)
    a_rt = (np.float32(1.0) - rt).astype(np.float32)
    return a_at, a_rt


def build_in_maps(signal, z_alpha):
    """Host-side prep: pad x (W+1 ones, one trailing zero), fp16 x and its
    first difference, per-partition coefficient vectors (chunk-major)."""
    signal = np.ascontiguousarray(np.asarray(signal, dtype=np.float32))
    a_at, a_rt = _coeffs(z_alpha)
    mid = ((a_at + a_rt) * np.float32(0.5)).astype(np.float32)
    hdl = ((a_rt - a_at) * np.float32(0.5)).astype(np.float32)
    dlt = (a_rt - a_at).astype(np.float32)

    pa = np.empty((B, TP), np.float32)
    pa[:, :W + 1] = 1.0
    pa[:, W + 1:W + 1 + T] = signal
    pa[:, TP - 1] = 0.0
    xh = pa.astype(np.float16)
    dx = (pa[:, 1:] - pa[:, :-1]).astype(np.float16)

    in_maps = []
    prow = np.arange(128) % RPC  # local row of each partition (chunk-major)
    for ci in range(NCORES):
        rows = slice(ci * RPC, (ci + 1) * RPC)
        sel = ci * RPC + prow
        in_maps.append({
            "xh": xh[rows],
            "dx": dx[rows],
            "mid": mid[sel][:, None].astype(np.float32),
            "hdl": hdl[sel][:, None].astype(np.float32),
            "aat": a_at[sel][:, None].astype(np.float32),
            "dlt": dlt[sel][:, None].astype(np.float32),
        })
    return in_maps


def kernel(signal, z_alpha):
    from concourse import bass_utils
    nc = _get_nc()
    in_maps = build_in_maps(signal, z_alpha)
    res = bass_utils.run_bass_kernel_spmd(nc, in_maps, core_ids=list(range(NCORES)))
    out = np.concatenate([r["y"] for r in res.results], axis=0)
    return out.astype(np.float32)


if __name__ == "__main__":
    rng = np.random.default_rng(0)
    sig = rng.standard_normal((B, T)).astype(np.float32)
    za = rng.standard_normal((B, 2)).astype(np.float32)
    y = kernel(sig, za)
    print("kernel ran:", y.shape, y.dtype)
